# revision 22
# baseline (speedup 1.0000x reference)
"""Trainium2 Bass kernel for the physics-informed MLP forecaster.

Model (per batch row of `history` [B, 24]):
  1. physics: 20-step delayed-feedback recurrence on the last history value
       T_new = (1-a)*T - b*T_delayed - g*T^3   (a,b = sigmoid(alpha/beta))
     with T_delayed from tau_int steps back (history first, then preds).
  2. x = [history(24) ; T_physics(20)] -> 3-layer tanh MLP (44->256^3)
     -> T_soft = c @ cor_w2 + cor_b2;  T_pred = T_physics + sigmoid(lm)*T_soft

Mapping (pure data parallel, 8 cores x 32768 rows; row = p*W + w on 128
partitions):
  * Physics runs on the DVE, step-major, in column chunks: a 32-col first
    chunk (fed by its own small htail DMA) unblocks the first MLP tiles at
    ~30us; the remaining chunks interleave with the MLP's DVE work. The
    exact fp32 state `pf` is DMA'd out step-major as soon as each half
    completes; the host untransposes it (free).
  * history ships fp16 from the host (it only feeds the fp16 MLP input);
    2x-mode DVE copies drop it into comb16. htail stays fp32 so the
    recurrence state is exact.
  * MLP is feature-major: per j-block the PE transposes comb16 [128,44]
    (fp16, 1 cyc/row) into PSUM; a DVE copy builds x^T [44,512] tiles.
    L1..L3 run fp16 matmuls (N=512); both M-halves share one 2-bank PSUM
    tile so tanh runs as ONE wide ACT op when biases are zero (the ACT
    engine at 1 elem/cycle/lane is the binding ~213us floor). L4 runs
    batch-major per j-block into a 4-tile PSUM accumulator, so soft/pred
    staging is 2 batched DVE ops per 4 tiles into the [.,40] output tile;
    4 skewed chunk DMAs stream it out.
  * This walrus build allows ONE sync-wait per instruction. A vector-clock
    transitive reduction over per-proc streams (engines + DMA queue rings)
    prunes implied waits -- but never same-engine RAW-visibility waits on
    DVE/Pool/SP, and never non-monotone barrier_* rendezvous sems. Surplus
    waits are demoted onto preceding same-engine drains (stalling issue,
    not execution), restricted to past-posted sems so the sequencer can
    never deadlock on work it hasn't issued. Multi-wait tail drains split
    into single-wait chains.
"""

import numpy as np

B = 262144
HIST = 24
FORE = 20
HID = 256
NCORES = 8
P = 128


def _build_nc(w, c1, bcoef, g, lam, tau_int, zero_bias=False):
    """Build the per-core Bass program. w = rows per partition (rows = 128*w)."""
    from contextlib import ExitStack

    import concourse.bass as bass
    import concourse.mybir as mybir
    import concourse.tile as tile

    f32 = mybir.dt.float32
    f16 = mybir.dt.float16
    AF = mybir.ActivationFunctionType
    ALU = mybir.AluOpType

    assert w % 4 == 0
    rows = P * w
    ntiles = w // 4  # 4 j-blocks (512 batch rows) per MLP tile

    nc = bass.Bass(trn_type="TRN2")

    WPK = HID + 2 * HID + 2 * HID + 2 * FORE + P  # w1 | w2 | w3 | w4 | ident16
    BPK = 6 + FORE + P  # b1|b2|b3 (2 cols each) | b4 broadcast | identity
    CA = w // 8  # first physics chunk width
    hist_d = nc.declare_dram_parameter("hist16", [rows, HIST], f16, isOutput=False)
    # delayed-history buffers, host-pregathered into the exact step-major
    # on-chip layout (one contiguous block each -> fast DMA, no DVE gather)
    hlA_d = nc.declare_dram_parameter("hlA", [P, tau_int * CA], f32, isOutput=False)
    hlB_d = nc.declare_dram_parameter(
        "hlB", [P, tau_int * (w - CA)], f32, isOutput=False
    )
    wpk_d = nc.declare_dram_parameter("wpk", [P, WPK], f16, isOutput=False)
    bpk_d = nc.declare_dram_parameter("bpk", [P, BPK], f32, isOutput=False)
    out_d = nc.declare_dram_parameter("out40", [rows, 40], f32, isOutput=True)
    # physics preds, step-major fp32; host untransposes. Two tensors (one
    # per chunk completion) so each DMA fires as soon as its half is done.
    W_DVE = w // 2
    physA_d = nc.declare_dram_parameter("physA", [P, FORE * W_DVE], f32, isOutput=True)
    physB_d = nc.declare_dram_parameter(
        "physB", [P, FORE * (w - W_DVE)], f32, isOutput=True
    )
    scr_d = nc.declare_dram_parameter("scr", [1, 16], f16, isOutput=True)

    CPAD = 128  # padded per-row MLP input stride (XBAR tile width)

    with ExitStack() as ctx:
        tc = ctx.enter_context(tile.TileContext(nc))
        const = ctx.enter_context(tc.tile_pool(name="const", bufs=1))
        xtp = ctx.enter_context(tc.tile_pool(name="xtp", bufs=3))
        hsb = ctx.enter_context(tc.tile_pool(name="hsb", bufs=3))
        pxp = ctx.enter_context(tc.tile_pool(name="pxp", bufs=1, space="PSUM"))
        php = ctx.enter_context(tc.tile_pool(name="php", bufs=1, space="PSUM"))
        spp = ctx.enter_context(tc.tile_pool(name="spp", bufs=1, space="PSUM"))

        hb16 = const.tile([P, w * HIST], f16)
        st = const.tile([P, w * 40], f32)
        # physics preds, batch-major fp32 (exact recurrence state)
        pf = const.tile([P, w * FORE], f32)
        # fp16 shadow of the combined MLP input [hist(24)|preds(20)|pad(84)]
        # per row; 128-wide rows so the DMA XBAR can transpose whole tiles
        comb16 = const.tile([P, w * CPAD], f16)
        wpkt = const.tile([P, WPK], f16)
        bpkt = const.tile([P, BPK], f32)

        # views into the packed parameter tiles
        NF = HIST + FORE  # 44 input features
        w1t = wpkt[0:NF, 0:HID]
        w2t = wpkt[:, HID : 3 * HID].rearrange("p (k m) -> p k m", k=2)
        w3t = wpkt[:, 3 * HID : 5 * HID].rearrange("p (k m) -> p k m", k=2)
        w4t = wpkt[:, 5 * HID : 5 * HID + 2 * FORE].rearrange(
            "p (k m) -> p k m", k=2
        )
        idt16 = wpkt[:, 5 * HID + 2 * FORE : 5 * HID + 2 * FORE + P]
        b1t = bpkt[:, 0:2]
        b2t = bpkt[:, 2:4]
        b3t = bpkt[:, 4:6]
        b4t = bpkt[:, 6 : 6 + FORE]
        idt = bpkt[:, 6 + FORE : 6 + FORE + P]

        # ---- input DMAs ----
        # hlast chunks are host-pregathered: chunk A lands first and alone
        # on its queue so the physics chain starts ASAP
        hlastA = const.tile([P, tau_int * CA], f32)
        hlastB = const.tile([P, tau_int * (w - CA)], f32)
        nc.sync.dma_start(out=hlastA, in_=hlA_d[:])
        nc.sync.dma_start(out=hlastB, in_=hlB_d[:])
        nc.sync.dma_start(out=hb16, in_=hist_d[:].rearrange("(p q) c -> p (q c)", p=P))
        nc.sync.dma_start(out=wpkt, in_=wpk_d[:])
        nc.sync.dma_start(out=bpkt, in_=bpk_d[:])

        # "Observe" pass: with a 1-sync-wait budget per instruction, each
        # engine observes the parameter DMAs once up front via a tiny op, so
        # real matmuls/activations/DVE ops never need DMA waits of their own.
        obs = spp.tile([1, P], f32, tag="sp")
        nc.tensor.transpose(obs[0:1, 0:P], idt[:, 0:1], idt)  # bpk (ident)
        nc.tensor.transpose(obs[0:1, 0:P], wpkt[:, 0:2].bitcast(f32), idt)
        obs_a = const.tile([1, 1], f32)
        obs_v = const.tile([1, 2], f32)
        nc.scalar.copy(obs_a[0:1, 0:1], bpkt[0:1, 0:1])
        nc.vector.tensor_copy(obs_v[0:1, 0:1], bpkt[0:1, 0:1])

        hb3 = hb16.rearrange("p (q c) -> p q c", c=HIST)
        st3 = st.rearrange("p (q c) -> p q c", c=40)
        cb16 = comb16.rearrange("p (q c) -> p q c", c=CPAD)
        out3 = out_d[:].rearrange("(p q) c -> p q c", p=P)

        # ---- physics recurrence (DVE), step-major contiguous layout ----
        # 3 column-chunks; the first (32 cols) unblocks the first 8 MLP
        # tiles early. hlast chunks arrive pre-gathered from the host.
        scr_u = const.tile([P, w], f32)
        scr_r = const.tile([P, w], f32)
        scr_s = const.tile([P, w], f32)

        def phys_step_ops(c0, wc, s):
            # the 4 DVE ops of one recurrence step, as closures
            if c0 < CA:
                hl, hw0, ho = hlastA, CA, c0
            else:
                hl, hw0, ho = hlastB, w - CA, c0 - CA
            if s == 0:
                T = hl[:, (tau_int - 1) * hw0 + ho : (tau_int - 1) * hw0 + ho + wc]
            else:
                T = pf[:, (s - 1) * w + c0 : (s - 1) * w + c0 + wc]
            if s < tau_int:
                Td = hl[:, s * hw0 + ho : s * hw0 + ho + wc]
            else:
                Td = pf[:, (s - tau_int) * w + c0 : (s - tau_int) * w + c0 + wc]
            u = scr_u[:, 0:wc]
            r = scr_r[:, 0:wc]
            t2 = scr_s[:, 0:wc]
            Tn = pf[:, s * w + c0 : s * w + c0 + wc]
            # u = T*T ; r = (u*g)*T = g*T^3 ; t2 = b*Td + r ; Tn = c1*T - t2
            return [
                lambda: nc.vector.tensor_tensor(out=u, in0=T, in1=T, op=ALU.mult),
                lambda: nc.vector.scalar_tensor_tensor(
                    out=r, in0=u, scalar=g, in1=T, op0=ALU.mult, op1=ALU.mult
                ),
                lambda: nc.vector.scalar_tensor_tensor(
                    out=t2, in0=Td, scalar=bcoef, in1=r, op0=ALU.mult, op1=ALU.add
                ),
                lambda: nc.vector.scalar_tensor_tensor(
                    out=Tn, in0=T, scalar=c1, in1=t2, op0=ALU.mult, op1=ALU.subtract
                ),
            ]

        def stage_ops(q0, wq):
            # stage cols [q0:q0+wq] into comb16: preds via transposed strided
            # copies ((s,q) step-major -> (q,s)), hist via 2x packed copies
            src_ap = bass.AP(
                tensor=pf.tensor,
                offset=pf.offset + q0,
                ap=[pf.ap[0], [1, wq], [w, FORE]],
            )
            return [
                lambda: nc.vector.tensor_copy(
                    cb16[:, q0 : q0 + wq, HIST : HIST + FORE], src_ap
                ),
                lambda: nc.vector.tensor_copy(
                    cb16[:, q0 : q0 + wq, 0:HIST], hb3[:, q0 : q0 + wq]
                ),
            ]

        pf3v = pf.rearrange("p (s q) -> p s q", q=w)
        phA3 = physA_d[:].rearrange("p (s q) -> p s q", q=W_DVE)
        phB3 = physB_d[:].rearrange("p (s q) -> p s q", q=w - W_DVE)

        # first (small) chunk emitted eagerly: it unblocks the first 8 MLP
        # tiles. The remaining chunks are NOT emitted here -- the DVE queue
        # is in-order, so a contiguous 80-op chain ahead of the MLP's
        # per-tile x^T copies head-of-line-blocks the whole pipeline.
        # Instead they become a pending op list dribbled out between tiles.
        for s in range(FORE):
            for op in phys_step_ops(0, CA, s):
                op()
        for op in stage_ops(0, CA):
            op()
        pending = []
        c0 = CA
        for wc in (W_DVE - CA, w - W_DVE):
            for s in range(FORE):
                pending += phys_step_ops(c0, wc, s)
            pending += stage_ops(c0, wc)
            c0 += wc
            if c0 == W_DVE:
                # physics output: exact step-major fp32 state; host reorders
                pending.append(
                    lambda: nc.sync.dma_start(out=phA3, in_=pf3v[:, :, 0:W_DVE])
                )
        assert c0 == w
        pending.append(lambda: nc.sync.dma_start(out=phB3, in_=pf3v[:, :, W_DVE:w]))
        pending.reverse()  # pop() from the front

        # ---- MLP over tiles of 4 j-blocks (512 batch rows) ----
        NB = 4 * P  # moving free dim
        # skewed output chunks; extra out chunks ride the in-order SWDGE
        # stream. Marks MUST be multiples of SPT: soft/pred staging runs
        # once per SPT tiles, and a chunk DMA may only cover fully-staged
        # regions. SPT=2 keeps the exposed tail chunk small.
        SPT = 2  # tiles per sp accumulator batch
        assert ntiles % SPT == 0

        def _snap(x):
            return min(ntiles, SPT * max(1, round(x * ntiles / SPT)))

        if ntiles >= 16:
            # many skewed chunks; last chunk = one 2-tile batch so the
            # exposed tail is minimal
            out_marks = {_snap(x) for x in (0.2, 0.4, 0.55, 0.7, 0.85)}
            out_marks |= {ntiles - SPT, ntiles}
        else:
            out_marks = {ntiles}
        out_done = [0]

        # initialize the px holes (partitions NF..64 and 64+NF..128) once
        # per ring buffer so the single full-partition x^T copy never reads
        # uninitialized PSUM (PSUM APs must start 32-aligned). The fills
        # ride the otherwise-idle ACT engine during the ramp -- free.
        for _ in range(3):
            px0 = pxp.tile([P, NB], f16, tag="px")
            nc.vector.memset(px0[32:64, :].bitcast(f32), 0.0)
            nc.vector.memset(px0[96:128, :].bitcast(f32), 0.0)

        for t in range(ntiles):
            px = pxp.tile([P, NB], f16, tag="px")
            for jl in range(4):
                j = 4 * t + jl
                # x^T block: [128, 44] f16 -> [44, 128] f16 in PSUM, written
                # twice (col groups 0 and 64) so both L1 row groups have
                # x^T on their own partitions; the pair runs concurrently
                # in the PE array (different col_grp)
                nc.tensor.transpose(
                    px[0:NF, jl * P : (jl + 1) * P],
                    comb16[:, j * CPAD : j * CPAD + NF],
                    idt16,
                )
                nc.tensor.transpose(
                    px[64 : 64 + NF, jl * P : (jl + 1) * P],
                    comb16[:, j * CPAD : j * CPAD + NF],
                    idt16,
                )
            # x^T copy PSUM->SBUF must run on the DVE (GPSIMD cannot access
            # PSUM, DMA cannot read PSUM, ACT is the bottleneck engine).
            # One op covers both partition ranges (DVE cost is free-dim
            # elems; partitions are parallel lanes).
            xt = xtp.tile([P, NB], f16, tag="xt")
            nc.vector.tensor_copy(xt[:, :], px[:, :])
            # PE observe of the DVE clock: absorbing the wait in this tiny
            # fp16 transpose lets the L1 matmuls issue wait-free (fp32 1x1
            # transposes cost ~233ns in LOW_HIGH mode; fp16 is ~4x cheaper).
            nc.tensor.transpose(
                px[0:1, 0:1], xt[0:1, 0:1], idt16[0:1, 0:1],
            )

            def layer(tag, lhsT_of, rhs_of, bias):
                pp = php.tile([P, 2 * NB], f32, tag=tag)
                for m in range(2):
                    for k, (lhsT, sstop) in enumerate(lhsT_of(m)):
                        nc.tensor.matmul(
                            pp[:, m * NB : (m + 1) * NB],
                            lhsT,
                            rhs_of(k),
                            start=(k == 0),
                            stop=sstop,
                        )
                ot = hsb.tile([P, 2 * NB], f16, tag=tag + "s")
                if zero_bias:
                    nc.scalar.activation(ot, pp, AF.Tanh)
                else:
                    for m in range(2):
                        nc.scalar.activation(
                            ot[:, m * NB : (m + 1) * NB],
                            pp[:, m * NB : (m + 1) * NB],
                            AF.Tanh,
                            bias=bias[:, m : m + 1],
                        )
                return ot

            # L1: four M=64 quadrant matmuls run CONCURRENTLY in the PE
            # array (row groups {0,64} x col groups {0,64}): w1 is
            # replicated at partitions 64.. (host prep) and each row group
            # streams its own x^T copy. Wall ~= one N=512 matmul instead
            # of two. Output layout in PSUM is identical to the m-loop's.
            if zero_bias:
                pp1 = php.tile([P, 2 * NB], f32, tag="h")
                for jq in range(4):
                    r = 0 if jq < 2 else 64
                    nc.tensor.matmul(
                        pp1[(jq % 2) * 64 : (jq % 2) * 64 + 64,
                            (jq // 2) * NB : (jq // 2 + 1) * NB],
                        wpkt[r : r + NF, jq * 64 : (jq + 1) * 64],
                        xt[r : r + NF, :],
                        start=True,
                        stop=True,
                    )
                htb = hsb.tile([P, 2 * NB], f16, tag="hs")
                nc.scalar.activation(htb, pp1, AF.Tanh)
            else:
                htb = layer(
                    "h",
                    lambda m: [(w1t[:, m * P : (m + 1) * P], True)],
                    lambda k: xt[0:NF, :],
                    b1t,
                )
            hts = [htb[:, 0:NB], htb[:, NB : 2 * NB]]
            ftb = layer(
                "f",
                lambda m: [
                    (w2t[:, 0, m * P : (m + 1) * P], False),
                    (w2t[:, 1, m * P : (m + 1) * P], True),
                ],
                lambda k: hts[k],
                b2t,
            )
            fts = [ftb[:, 0:NB], ftb[:, NB : 2 * NB]]
            ctb = layer(
                "c",
                lambda m: [
                    (w3t[:, 0, m * P : (m + 1) * P], False),
                    (w3t[:, 1, m * P : (m + 1) * P], True),
                ],
                lambda k: fts[k],
                b3t,
            )
            cts = [ctb[:, 0:NB], ctb[:, NB : 2 * NB]]

            # L4 batch-major per j-block into a 4-tile PSUM accumulator:
            # T_soft[128,20] = (c^T block).T @ w4. After SPT tiles, soft/pred
            # staging runs as 2 batched DVE ops.
            ti = t % SPT
            if ti == 0:
                sp = spp.tile([P, SPT * 4 * FORE], f32, tag="sp")
            for jl in range(4):
                for k in range(2):
                    nc.tensor.matmul(
                        sp[:, (ti * 4 + jl) * FORE : (ti * 4 + jl + 1) * FORE],
                        cts[k][:, jl * P : (jl + 1) * P],
                        w4t[:, k, :],
                        start=(k == 0),
                        stop=(k == 1),
                    )
            if ti == SPT - 1:
                t0 = t - (SPT - 1)
                q0 = 4 * t0
                nq = 4 * SPT
                sp3 = sp.rearrange("p (q c) -> p q c", c=FORE)
                soft = st3[:, q0 : q0 + nq, 0:FORE]
                pred = st3[:, q0 : q0 + nq, FORE : 2 * FORE]
                # phys batch-major view from the step-major pf: (q, s)
                pf_qs = bass.AP(
                    tensor=pf.tensor,
                    offset=pf.offset + q0,
                    ap=[pf.ap[0], [1, nq], [w, FORE]],
                )
                if zero_bias:
                    nc.vector.tensor_copy(soft, sp3)
                else:
                    b4b = b4t.unsqueeze(1).broadcast_to((P, nq, FORE))
                    nc.vector.tensor_tensor(out=soft, in0=sp3, in1=b4b, op=ALU.add)
                nc.vector.scalar_tensor_tensor(
                    out=pred, in0=sp3 if zero_bias else soft, scalar=lam,
                    in1=pf_qs, op0=ALU.mult, op1=ALU.add,
                )

            # chunked output DMAs (SWDGE: issued from the idle gpsimd
            # sequencer, so they ride the in-order Pool stream and the HW
            # rings stay XBAR-only)
            if (t + 1) in out_marks:
                q0 = out_done[0]
                nc.sync.dma_start(
                    out=out3[:, 4 * q0 : 4 * (t + 1), :],
                    in_=st3[:, 4 * q0 : 4 * (t + 1), :],
                )
                out_done[0] = t + 1

            # dribble the remaining physics chain between tiles so the
            # in-order DVE queue never head-of-line-blocks the MLP stream.
            # Chunk B (cols CA..W_DVE, feeds tiles 8..) drains by ~tile 7;
            # chunk C (feeds tiles 32..) by ~tile 22.
            drain = 12 if t < 8 else 6
            for _ in range(min(drain, len(pending))):
                pending.pop()()

        assert not pending

    _prune_redundant_waits(nc)
    _demote_extra_waits(nc)
    _split_fat_drains(nc)
    return nc


def _demote_extra_waits(nc):
    """Move surplus waits off multi-wait instructions onto preceding
    same-engine InstDrains.

    The issuing sequencer executes the drain (stalling until the surplus
    condition holds) before enqueuing/executing the instruction itself, so
    the instruction retires with a single wait. Conservative: only delays
    issue. Engine-sem waits are kept on the instruction (they pace real
    data deps); DMA-queue sems get demoted first.
    """
    import concourse.mybir as mybir

    fn = nc.m.functions[0]
    # A demoted wait stalls the issuing sequencer BEFORE this instruction,
    # so it may only reference semaphore values posted by instructions that
    # appear EARLIER in the program — otherwise the sequencer deadlocks
    # waiting on work it hasn't issued yet. Precompute, per sem, the
    # cumulative post value at each program position.
    insts = [i for bb in fn.blocks for i in bb.instructions]
    sem_pos: dict[str, list[tuple[int, int]]] = {}  # sem -> [(cum, pos)]
    cum: dict[str, int] = {}
    from concourse.tile_sem_assignment import PROC_NAME_TO_IDX

    idx_to_proc = {v: k for k, v in PROC_NAME_TO_IDX.items()}
    for pos, inst in enumerate(insts):
        si = inst.sync_info
        if si and si.on_update:
            for u in si.on_update:
                nm = getattr(u, "ant_name", None)
                if nm:
                    cum[nm] = cum.get(nm, 0) + getattr(u, "update_value", 1)
                    sem_pos.setdefault(nm, []).append((cum[nm], pos))

    def posted_by(nm, v):
        for pv, pos in sem_pos.get(nm, []):
            if pv >= v:
                return pos
        return 1 << 60

    pos_of = {id(i): p for p, i in enumerate(insts)}
    for bb in fn.blocks:
        il = bb.instructions
        idx = 0
        while idx < len(il):
            inst = il[idx]
            si = inst.sync_info
            if (
                not isinstance(inst, mybir.InstDrain)
                and si
                and si.on_wait
                and len(si.on_wait) > 1
            ):
                mypos = pos_of.get(id(inst), 1 << 60)
                waits = list(si.on_wait)
                # demote only past-posted waits on monotone (engine/queue)
                # sems; future-posted or protocol sems must stay on the
                # instruction (evaluated at its ring/engine, not at issue)
                import re

                _MOD = re.compile(r"^(PE|DVE|Activation|Pool|SP|DMAHW\d|DMASW\d)_")
                demotable = [
                    wt for wt in waits
                    if wt.ant_name and _MOD.match(wt.ant_name)
                    and posted_by(wt.ant_name, wt.wait_value) < mypos
                ]
                keep = [wt for wt in waits if wt not in demotable]
                if not keep:
                    # everything is past-posted: keep the largest-value one
                    demotable.sort(key=lambda wt: wt.wait_value)
                    keep = [demotable.pop()]
                assert len(keep) == 1, (
                    f"{inst.name}: {len(keep)} future-posted waits, cannot "
                    f"reduce to one: {[(w.ant_name, w.wait_value) for w in waits]}"
                )
                for j, wt in enumerate(demotable):
                    d = mybir.InstDrain(
                        name=f"{inst.name}-dw{j}", ins=[], outs=[]
                    )
                    d.engine = inst.engine
                    d.sync_info = mybir.SyncInfo(on_wait=[wt], on_update=[])
                    try:
                        nc.register_instruction(d, overwrite=True)
                    except Exception:
                        pass
                    il.insert(idx, d)
                    idx += 1
                si.on_wait = keep
            idx += 1


def _split_fat_drains(nc):
    """Split multi-wait drains into chains of single-wait drains.

    Every instruction struct in this walrus build accepts one sync wait;
    the Tile kernel-tail drain gathers all procs on one instruction. A
    sequence of drains on the same in-order queue is semantically
    identical.
    """
    import concourse.mybir as mybir

    fn = nc.m.functions[0]
    for bb in fn.blocks:
        il = bb.instructions
        idx = 0
        while idx < len(il):
            inst = il[idx]
            si = inst.sync_info
            if (
                isinstance(inst, mybir.InstDrain)
                and si
                and si.on_wait
                and len(si.on_wait) > 1
            ):
                waits = list(si.on_wait)
                for j, wt in enumerate(waits[:-1]):
                    d = mybir.InstDrain(name=f"{inst.name}-w{j}", ins=[], outs=[])
                    d.engine = inst.engine
                    d.sync_info = mybir.SyncInfo(on_wait=[wt], on_update=[])
                    try:
                        nc.register_instruction(d, overwrite=True)
                    except Exception:
                        pass
                    il.insert(idx, d)
                    idx += 1
                si.on_wait = [waits[-1]]
            idx += 1


def _prune_redundant_waits(nc):
    """Vector-clock transitive reduction of semaphore waits.

    This walrus build allows ONE sync wait per instruction. Model every
    in-order execution stream as a proc: each engine sequencer, and each
    DMA queue ring (DMAHW0..7 / DMASW0..7, in-order descriptor
    processing). Each proc accumulates a monotone clock of semaphore
    values it is known to have observed: its own posts (in-order
    completion), plus — for every wait it keeps — the posting
    instruction's clock snapshot at the waited value (happens-before).
    A wait already implied by the proc's clock is dropped.
    """
    import concourse.mybir as mybir
    from concourse.tile_sem_assignment import PROC_NAME_TO_IDX

    idx_to_proc = {v: k for k, v in PROC_NAME_TO_IDX.items()}
    fn = nc.m.functions[0]
    insts = [i for bb in fn.blocks for i in bb.instructions]

    def proc_of(inst):
        sp = getattr(inst, "bass_scheduled_proc", None)
        if isinstance(sp, int) and sp in idx_to_proc:
            nm = idx_to_proc[sp]
            if nm.startswith("DMA"):
                return nm
        return str(inst.engine)

    import re

    # Only engine-completion and DMA-queue sems are monotone counters the
    # model understands. Anything else (barrier_* rendezvous sems get RESET
    # between uses) must never be pruned nor trusted for implication.
    _MODELED = re.compile(r"^(PE|DVE|Activation|Pool|SP|DMAHW\d|DMASW\d)_")

    clocks: dict[str, dict[str, int]] = {}
    # sem -> list of (cum_value, snapshot dict) in posting order
    posts: dict[str, list[tuple[int, dict]]] = {}
    cum: dict[str, int] = {}
    pruned = 0
    for inst in insts:
        si = inst.sync_info
        proc = proc_of(inst)
        know = clocks.setdefault(proc, {})
        # An engine's own-sem waits protect same-engine RAW hazards (issue
        # order does not imply write visibility). PE reads only SBUF and
        # writes only PSUM, ACT never reads its own output, so their
        # self-waits are safe to prune; DVE/Pool/SP self-waits must stay.
        own_pfx = None
        if proc.startswith("EngineType."):
            eng_nm = proc.split(".", 1)[1]
            if eng_nm not in ("PE", "Activation"):
                own_pfx = eng_nm + "_"
        if si and si.on_wait:
            keep = []
            for wt in si.on_wait:
                nm = wt.ant_name
                v = wt.wait_value
                modeled = bool(nm and _MODELED.match(nm))
                self_raw = bool(nm and own_pfx and nm.startswith(own_pfx))
                if modeled and not self_raw and know.get(nm, 0) >= v:
                    pruned += 1
                    continue
                keep.append(wt)
                if modeled:
                    # merge the poster's snapshot at the first post >= v
                    for pv, snap in posts.get(nm, []):
                        if pv >= v:
                            for s2, v2 in snap.items():
                                if know.get(s2, 0) < v2:
                                    know[s2] = v2
                            break
                    if know.get(nm, 0) < v:
                        know[nm] = v
            if len(keep) != len(si.on_wait):
                si.on_wait = keep
        if si and si.on_update:
            for u in si.on_update:
                nm = getattr(u, "ant_name", None)
                if nm and _MODELED.match(nm):
                    cum[nm] = cum.get(nm, 0) + getattr(u, "update_value", 1)
                    if know.get(nm, 0) < cum[nm]:
                        know[nm] = cum[nm]
                    posts.setdefault(nm, []).append((cum[nm], dict(know)))
    return pruned


def _prep_weights(enc_w1, enc_b1, enc_w2, enc_b2, cor_w1, cor_b1, cor_w2, cor_b2):
    f32, f16 = np.float32, np.float16
    WPK = HID + 2 * HID + 2 * HID + 2 * FORE + P
    wpk = np.zeros((P, WPK), f16)
    wpk[:, 5 * HID + 2 * FORE : 5 * HID + 2 * FORE + P] = np.eye(P, dtype=f16)
    wpk[0 : HIST + FORE, 0:HID] = enc_w1.astype(f16)
    # replica at partitions 64.. for the L1 quadrant matmuls (row group 64)
    wpk[64 : 64 + HIST + FORE, 0:HID] = enc_w1.astype(f16)
    wpk[:, HID : 3 * HID] = (
        enc_w2.reshape(2, P, HID).transpose(1, 0, 2).reshape(P, 2 * HID).astype(f16)
    )
    wpk[:, 3 * HID : 5 * HID] = (
        cor_w1.reshape(2, P, HID).transpose(1, 0, 2).reshape(P, 2 * HID).astype(f16)
    )
    wpk[:, 5 * HID : 5 * HID + 2 * FORE] = (
        cor_w2.reshape(2, P, FORE).transpose(1, 0, 2).reshape(P, 2 * FORE).astype(f16)
    )
    BPK = 6 + FORE + P
    bpk = np.zeros((P, BPK), f32)
    bpk[:, 0:2] = enc_b1.reshape(2, P).T
    bpk[:, 2:4] = enc_b2.reshape(2, P).T
    bpk[:, 4:6] = cor_b1.reshape(2, P).T
    bpk[:, 6 : 6 + FORE] = np.broadcast_to(cor_b2.reshape(1, FORE), (P, FORE))
    bpk[:, 6 + FORE : 6 + FORE + P] = np.eye(P, dtype=f32)
    return dict(wpk=wpk, bpk=bpk)


LAST_RESULT = None  # BassKernelResults of the most recent kernel() call


def kernel(history, enc_w1, enc_b1, enc_w2, enc_b2, cor_w1, cor_b1, cor_w2, cor_b2,
           alpha, beta, gamma, tau, lambda_mix):
    from concourse.bass_utils import run_bass_kernel_spmd

    global LAST_RESULT

    history = np.asarray(history, np.float32)
    assert history.shape == (B, HIST)

    def sig(x):
        return float(1.0 / (1.0 + np.exp(-np.float64(x))))

    a = sig(alpha)
    bcoef = sig(beta)
    g = float(abs(np.float64(gamma)))
    lam = sig(lambda_mix)
    c1 = 1.0 - a
    tau_int = int(np.clip(float(tau), 1.0, 18.0))

    zb = not (
        np.any(np.asarray(enc_b1)) or np.any(np.asarray(enc_b2))
        or np.any(np.asarray(cor_b1))
    )
    w = B // NCORES // P  # rows per partition per core
    nc = _build_nc(w, c1, bcoef, g, lam, tau_int, zero_bias=zb)

    shared = _prep_weights(
        np.asarray(enc_w1, np.float32), np.asarray(enc_b1, np.float32),
        np.asarray(enc_w2, np.float32), np.asarray(enc_b2, np.float32),
        np.asarray(cor_w1, np.float32), np.asarray(cor_b1, np.float32),
        np.asarray(cor_w2, np.float32), np.asarray(cor_b2, np.float32),
    )
    rows = B // NCORES
    hist16_full = history.astype(np.float16)
    # host-pregathered delayed-history chunks in the exact on-chip
    # step-major layout: hl[p, s*wq + q] = history[p*w + q, HIST-tau+s]
    CA = w // 8
    in_maps = []
    for i in range(NCORES):
        hc = history[i * rows : (i + 1) * rows].reshape(P, w, HIST)
        htail = hc[:, :, HIST - tau_int :]  # [P, w, tau]
        hlA = np.ascontiguousarray(htail[:, 0:CA].transpose(0, 2, 1)).reshape(
            P, tau_int * CA
        )
        hlB = np.ascontiguousarray(htail[:, CA:w].transpose(0, 2, 1)).reshape(
            P, tau_int * (w - CA)
        )
        in_maps.append(
            {
                "hist16": hist16_full[i * rows : (i + 1) * rows],
                "hlA": hlA,
                "hlB": hlB,
                **shared,
            }
        )

    res = run_bass_kernel_spmd(nc, in_maps, core_ids=list(range(NCORES)))
    LAST_RESULT = res

    preds, physs, softs = [], [], []
    wpp = rows // P
    w2 = wpp // 2
    for i in range(NCORES):
        o = np.asarray(res.results[i]["out40"], np.float32).reshape(rows, 40)
        softs.append(o[:, 0:FORE])
        preds.append(o[:, FORE : 2 * FORE])
        pa = np.asarray(res.results[i]["physA"], np.float32).reshape(P, FORE, w2)
        pb = np.asarray(res.results[i]["physB"], np.float32).reshape(
            P, FORE, wpp - w2
        )
        ph = np.concatenate([pa, pb], axis=2)
        physs.append(ph.transpose(0, 2, 1).reshape(rows, FORE))
    T_soft = np.concatenate(softs, 0)
    T_pred = np.concatenate(preds, 0)
    T_physics = np.concatenate(physs, 0)
    return (T_pred, T_physics, T_soft)

